# revision 22
# baseline (speedup 1.0000x reference)
"""DGL DigitCapsuleLayer (dynamic routing, 3 iters) on 8 trn2 NeuronCores.

Strategy: data-parallel over batch B=256 (32 per core), W replicated.
u_hat ([1152,10,256,16] = 189MB) is NEVER materialized; both routing
contractions are computed as fused matmul pipelines from x and W:

  s[j,b,d]  = sum_{i,c} (c[i,j] * W[i,j,d,c]) * x[b,c,i]      (PE, 72 matmuls)
  t[i,j]    = sum_{c,d} W[i,j,d,c] * y[j,d,c,i],
  y[j,d,c,i]= sum_b v[j,b,d] * x[b,c,i]                        (PE + DVE reduce)

All big tensors keep i on the partition axis (i = k*128 + p, k in 0..8), so
softmax / the c*W product / the (c,d)-reduction are pure free-dim ops --
no cross-partition transpose anywhere.  b_ij updates need 2 AllReduces of
the [1152,10] agreement matrix (iter-1 softmax is uniform; the post-iter-3
update is dead code).
"""

import sys

for _p in ("/opt/trn_rl_repo",):
    if _p not in sys.path:
        sys.path.insert(0, _p)

import numpy as np

import concourse.bacc as bacc
import concourse.bass as bass
import concourse.mybir as mybir
import concourse.tile as tile
from concourse.bass_utils import run_bass_kernel_spmd

F32 = mybir.dt.float32
BF16 = mybir.dt.bfloat16
AX = mybir.AxisListType
ALU = mybir.AluOpType
ACTF = mybir.ActivationFunctionType

NCORES = 8
B = 256
BL = B // NCORES          # 32 batch rows per core
I = 1152                  # input capsules
K = I // 128              # 9 i-chunks of 128
P = 128
C = 8                     # in_dim
J = 10                    # out capsules
D = 16                    # out_dim
JD = J * D                # 160
KCJD = K * C * J * D      # 11520

_PROG = None


def _build_program():
    nc = bacc.Bacc(
        "TRN2",
        target_bir_lowering=False,
        debug=False,
        num_devices=NCORES,
    )

    G = K * C // 4            # 18 rounds of 4 PE tiles

    xs_d = nc.declare_dram_parameter("xs", [P, C, K, BL], F32, isOutput=False)
    xt_d = nc.declare_dram_parameter("xt", [P, G, P], F32, isOutput=False)
    wp_d = nc.declare_dram_parameter("wp", [P, C, K, J, D], F32, isOutput=False)
    eye_d = nc.declare_dram_parameter("eye", [P, P], F32, isOutput=False)
    vout_d = nc.declare_dram_parameter("vout", [BL, J, D], F32, isOutput=True)

    with tile.TileContext(nc) as tc:
        with (
            tc.tile_pool(name="sb", bufs=1) as sb,
            tc.tile_pool(name="ps", bufs=2, space="PSUM") as ps,
            tc.tile_pool(name="dram", bufs=1, space="DRAM") as dram,
        ):
            # ---- persistent SBUF tensors ----
            w1h = sb.tile([P, C, K, J, D], BF16)          # W, i-partitioned
            x1h = sb.tile([P, C, K, BL], BF16)            # lhsT for s-matmuls
            x2h = sb.tile([P, G, P], BF16)                # lhsT strips for T1
            cw = sb.tile([P, C, K, J, D], BF16)           # c[i,j] * W
            yh = sb.tile([P, C, K, J, D], BF16)           # y, i-partitioned
            ph = sb.tile([P, C, K, J, D], BF16, tag="bigscratch")  # W .* y
            eye_sb = sb.tile([P, P], F32)                 # col-tile combiner
            sfull = sb.tile([P, JD], F32)                 # col-tile partials
            chd = sb.tile([P, K, J, D], BF16)             # c bcast over d
            w1h_f = w1h.rearrange("p c k j d -> p (c k j d)")
            yh_f = yh.rearrange("p c k j d -> p (c k j d)")
            ph_f = ph.rearrange("p c k j d -> p (c k j d)")

            # ---- load + cast (fp32 DMA staging -> bf16) ----
            # x / eye first (small; unblocks it1's s-phase early), then the W
            # quarters. Disjoint staging ranges: dynamic HWDGE DMAs only
            # support one sync-wait slot, so no buffer reuse here.
            nc.sync.dma_start(eye_sb[:, :], eye_d[:, :])

            x1_stage = sb.tile([P, K * C * BL], F32)
            nc.sync.dma_start(x1_stage[:, :], xs_d.rearrange("p c k b -> p (c k b)"))
            nc.scalar.copy(x1h.rearrange("p c k b -> p (c k b)"), x1_stage[:, :])

            w_stage = sb.tile([P, KCJD], F32, tag="bigscratch")
            NQ = 4
            QW = KCJD // NQ
            for q in range(NQ):
                nc.sync.dma_start(
                    w_stage[:, q * QW:(q + 1) * QW],
                    wp_d.rearrange("p c k j d -> p (c k j d)")[:, q * QW:(q + 1) * QW],
                )
                lo, mid, hi = q * QW, q * QW + (QW * 5) // 8, (q + 1) * QW
                nc.vector.tensor_copy(w1h_f[:, lo:mid], w_stage[:, lo:mid])
                nc.scalar.copy(w1h_f[:, mid:hi], w_stage[:, mid:hi])

            x2_stage = sb.tile([P, G * P], F32)
            nc.sync.dma_start(x2_stage[:, :], xt_d.rearrange("p g q -> p (g q)"))
            nc.scalar.copy(x2h.rearrange("p g q -> p (g q)"), x2_stage[:, :])

            # ---- DRAM bounce buffers for the 2 AllReduces ----
            t_b1 = dram.tile([P, K * J], BF16)
            t_b2 = dram.tile([P, K * J], BF16)
            t_ar1 = dram.tile([P, K * J], BF16)
            t_ar2 = dram.tile([P, K * J], BF16)

            # ---- small SBUF working tensors ----
            bsum = sb.tile([P, K, J], F32)     # running sum of AllReduced t

            def softmax_to_cw():
                """c = softmax(bsum/256, axis=j); cw = c * W (bf16)."""
                mrow = sb.tile([P, K], F32, tag="smx_m")
                nc.vector.tensor_reduce(mrow[:, :], bsum[:, :, :], axis=AX.X,
                                        op=ALU.max)
                bm = sb.tile([P, K, J], F32, tag="smx_bm")
                nc.vector.tensor_sub(
                    bm[:, :, :], bsum[:, :, :],
                    mrow.unsqueeze(2).broadcast_to([P, K, J]),
                )
                ex = sb.tile([P, K, J], F32, tag="smx_ex")
                nc.scalar.activation(ex[:, :, :], bm[:, :, :], ACTF.Exp,
                                     scale=1.0 / 256.0)
                ssum = sb.tile([P, K], F32, tag="smx_s")
                nc.vector.tensor_reduce(ssum[:, :], ex[:, :, :], axis=AX.X,
                                        op=ALU.add)
                rs = sb.tile([P, K], F32, tag="smx_r")
                nc.vector.reciprocal(rs[:, :], ssum[:, :])
                ch = sb.tile([P, K, J], F32, tag="smx_c")
                nc.vector.tensor_mul(
                    ch[:, :, :], ex[:, :, :],
                    rs.unsqueeze(2).broadcast_to([P, K, J]),
                )
                # chd[p,k,j,d] = c (materialized over d so the 8 per-c
                # products below hit the DVE 2x bf16 mode: step-1 innermost)
                nc.scalar.copy(
                    chd[:, :, :, :],
                    ch.unsqueeze(3).broadcast_to([P, K, J, D]),
                )
                for c in range(C):
                    nc.vector.tensor_mul(
                        cw[:, c, :, :, :].rearrange("p k j d -> p k (j d)"),
                        w1h[:, c, :, :, :].rearrange("p k j d -> p k (j d)"),
                        chd.rearrange("p k j d -> p k (j d)"),
                    )

            def s_phase(rhs_tile):
                """72 matmuls on 4 concurrent PE col-tiles, partials combined
                (and replicated x4 for T1's row tiles) by the eye matmul."""
                scol = ps.tile([P, 4, 512], F32, tag="ps4", name="scol")
                for kk in range(K * C):
                    c, k = divmod(kk, K)
                    ct, g = kk % 4, kk // 4
                    nc.tensor.matmul(
                        scol[32 * ct:32 * ct + 32, ct, 0:JD],
                        lhsT=x1h[:, c, k, :],
                        rhs=rhs_tile[:, c, k, :, :].rearrange("p j d -> p (j d)"),
                        start=(g == 0),
                        stop=(g == G - 1),
                        tile_position=(0, 32 * ct),
                    )
                for ct in range(4):
                    eng = nc.scalar if ct % 2 == 0 else nc.vector
                    if ct % 2 == 0:
                        nc.scalar.copy(sfull[32 * ct:32 * ct + 32, :],
                                       scol[32 * ct:32 * ct + 32, ct, 0:JD])
                    else:
                        nc.vector.tensor_copy(sfull[32 * ct:32 * ct + 32, :],
                                              scol[32 * ct:32 * ct + 32, ct, 0:JD])
                s2 = ps.tile([P, 4, 512], F32, tag="ps4", name="s2")
                nc.tensor.matmul(
                    s2[:, 0, 0:JD],
                    lhsT=eye_sb[:, :],
                    rhs=sfull[:, :],
                    start=True,
                    stop=True,
                )
                return s2[:, 0, 0:JD]

            def squash(s_ps, uniform, out_tile):
                """out_tile[p, j, d] = squash(scale * s); s is x4-replicated
                across the partition row-groups, so out is too.  Uses
                squash(s) = s * sqrt(|s|^2) / (1 + |s|^2)."""
                scale = 0.1 if uniform else 1.0
                s_sb = sb.tile([P, J, D], F32, tag="sq_s")
                nc.scalar.activation(s_sb.rearrange("p j d -> p (j d)"), s_ps,
                                     ACTF.Copy, scale=scale)
                ssq = sb.tile([P, J, D], F32, tag="sq_ssq")
                nc.vector.tensor_mul(ssq[:, :, :], s_sb[:, :, :], s_sb[:, :, :])
                sq = sb.tile([P, J], F32, tag="sq_sq")
                nc.vector.tensor_reduce(sq[:, :], ssq[:, :, :], axis=AX.X,
                                        op=ALU.add)
                rt = sb.tile([P, J], F32, tag="sq_rt")
                nc.scalar.activation(rt[:, :], sq[:, :], ACTF.Sqrt)
                sp1 = sb.tile([P, J], F32, tag="sq_sp1")
                nc.vector.tensor_scalar_add(sp1[:, :], sq[:, :], 1.0)
                fi = sb.tile([P, J], F32, tag="sq_fi")
                nc.vector.reciprocal(fi[:, :], sp1[:, :])
                g = sb.tile([P, J], F32, tag="sq_g")
                nc.vector.tensor_mul(g[:, :], rt[:, :], fi[:, :])
                nc.vector.tensor_mul(
                    out_tile[:, :, :],
                    s_sb[:, :, :],
                    g.unsqueeze(2).broadcast_to([P, J, D]),
                )

            def t_phase(vh4, t_bounce, t_arout, it):
                """T1 on 4 concurrent PE row-tiles; T2 = mul + bf16 add-tree;
                AllReduce of the [1152,10] agreement matrix."""
                vh4_f = vh4.rearrange("p j d -> p (j d)")
                for g in range(G):
                    yp = ps.tile([P, 4, 512], F32, tag="ps4", name=f"yp{it}_{g}")
                    for r in range(4):
                        nc.tensor.matmul(
                            yp[:, r, 0:JD],
                            lhsT=x2h[32 * r:32 * r + 32, g, :],
                            rhs=vh4_f[32 * r:32 * r + 32, :],
                            start=True,
                            stop=True,
                            tile_position=(32 * r, 0),
                        )
                    rng = slice(g * 4 * JD, (g + 1) * 4 * JD)
                    if g % 4 != 3:
                        nc.scalar.copy(yh_f[:, rng], yp[:, :, 0:JD])
                    else:
                        nc.vector.tensor_copy(yh_f[:, rng], yp[:, :, 0:JD])
                    if g == 8:
                        # W .* y for evacuated rounds overlaps remaining evac
                        nc.vector.tensor_mul(ph_f[:, 0:KCJD // 2],
                                             w1h_f[:, 0:KCJD // 2],
                                             yh_f[:, 0:KCJD // 2])
                    elif g == 13:
                        q3 = slice(KCJD // 2, (KCJD * 3) // 4)
                        nc.vector.tensor_mul(ph_f[:, q3], w1h_f[:, q3],
                                             yh_f[:, q3])
                q4 = slice((KCJD * 3) // 4, KCJD)
                nc.vector.tensor_mul(ph_f[:, q4], w1h_f[:, q4], yh_f[:, q4])
                # t[p, k, j] = sum_{c, d} P[p, c, k, j, d]: c-major layout
                # makes the c tree pure flat halves (DVE 2x bf16 mode).
                HALF = KCJD // 2
                tA = sb.tile([P, HALF], BF16, tag="treeA")
                nc.vector.tensor_add(tA[:, :], ph_f[:, 0:HALF],
                                     ph_f[:, HALF:KCJD])
                tB = sb.tile([P, HALF // 2], BF16, tag="treeB")
                nc.vector.tensor_add(tB[:, :], tA[:, 0:HALF // 2],
                                     tA[:, HALF // 2:HALF])
                tC = sb.tile([P, K, J, D], BF16, tag="treeC")
                nc.vector.tensor_add(
                    tC.rearrange("p k j d -> p (k j d)"),
                    tB[:, 0:HALF // 4], tB[:, HALF // 4:HALF // 2])
                tD = sb.tile([P, K, J, 8], BF16, tag="treeD")
                nc.vector.tensor_add(tD[:, :, :, :], tC[:, :, :, 0:8],
                                     tC[:, :, :, 8:16])
                tE = sb.tile([P, K, J, 4], BF16, tag="treeE")
                nc.vector.tensor_add(tE[:, :, :, :], tD[:, :, :, 0:4],
                                     tD[:, :, :, 4:8])
                tF = sb.tile([P, K, J, 2], BF16, tag="treeF")
                nc.vector.tensor_add(tF[:, :, :, :], tE[:, :, :, 0:2],
                                     tE[:, :, :, 2:4])
                tloc = sb.tile([P, K, J], BF16, name=f"tloc{it}")
                nc.vector.tensor_add(tloc[:, :, :], tF[:, :, :, 0],
                                     tF[:, :, :, 1])
                nc.gpsimd.dma_start(
                    t_bounce[:, :], tloc.rearrange("p k j -> p (k j)")
                )
                nc.gpsimd.collective_compute(
                    "AllReduce",
                    ALU.add,
                    replica_groups=[list(range(NCORES))],
                    ins=[t_bounce[:, :]],
                    outs=[t_arout[:, :]],
                )

            def ag_sum(ag_out, target, it):
                """Load the AllReduced [128, 90] sum into target (f32)."""
                st = sb.tile([P, K * J], BF16, tag=f"agst{it}")
                nc.gpsimd.dma_start(st[:, :], ag_out[:, :])
                nc.vector.tensor_copy(
                    target.rearrange("p k j -> p (k j)"), st[:, :])

            # ================= iteration 1 (c uniform = 1/10) =================
            v1 = sb.tile([P, J, D], BF16)
            s_ps = s_phase(w1h)
            squash(s_ps, True, v1)
            t_phase(v1, t_b1, t_ar1, 1)
            ag_sum(t_ar1, bsum, 1)

            # ================= iteration 2 =================
            softmax_to_cw()
            v2 = sb.tile([P, J, D], BF16)
            s_ps = s_phase(cw)
            squash(s_ps, False, v2)
            t_phase(v2, t_b2, t_ar2, 2)
            tsum2 = sb.tile([P, K, J], F32)
            ag_sum(t_ar2, tsum2, 2)
            nc.vector.tensor_add(bsum[:, :, :], bsum[:, :, :], tsum2[:, :, :])

            # ================= iteration 3 (no b update needed) ===============
            softmax_to_cw()
            v3 = sb.tile([P, J, D], F32)
            s_ps = s_phase(cw)
            squash(s_ps, False, v3)
            nc.gpsimd.dma_start(vout_d[:, :, :], v3[0:BL, :, :])

    nc.compile()
    return nc


def _get_program():
    global _PROG
    if _PROG is None:
        _PROG = _build_program()
    return _PROG


def _prep_inputs(x, W):
    """Host-side shard + layout permutation (pure reshapes, fp32 preserved)."""
    x = np.asarray(x, dtype=np.float32)
    W = np.asarray(W, dtype=np.float32)
    # wp[p, c, k, j, d] = W[k*128+p, j, d, c]  (c-major chunk order so the
    # T2 c-reduction is a flat halves tree)
    wp = np.ascontiguousarray(
        W.reshape(K, P, J, D, C).transpose(1, 4, 0, 2, 3))
    # eye[p, m] = 1 iff p == m (mod 32): the col-tile partial combiner,
    # which also replicates s across the 4 row-groups for T1's row tiles.
    eye = np.tile(np.eye(32, dtype=np.float32), (4, 4))
    in_maps = []
    for core in range(NCORES):
        xsh = x[core * BL:(core + 1) * BL]          # [BL, C, I]
        # xs[p, c, k, b] = xsh[b, c, k*128+p]
        xs = np.ascontiguousarray(
            xsh.transpose(2, 1, 0).reshape(K, P, C, BL).transpose(1, 2, 0, 3))
        # xt[32*(kk%4)+b, kk//4, p] = xsh[b, c, k*128+p] for kk=(c*K+k):
        # lhsT strips for the 4 concurrent T1 row-tiles.
        xt = np.empty((P, K * C // 4, P), dtype=np.float32)
        for kk in range(K * C):
            c, k = divmod(kk, K)
            r, g = kk % 4, kk // 4
            xt[32 * r:32 * r + 32, g, :] = xsh[:, c, k * P:(k + 1) * P]
        in_maps.append({"xs": xs, "xt": xt, "wp": wp, "eye": eye})
    return in_maps


def _ensure_ntff_hook():
    """Register the axon NTFF profile hook if the image's antenv lacks it."""
    import types

    try:
        from antenv.axon_hooks import get_axon_ntff_profile_hook  # noqa: F401
        return
    except ImportError:
        pass
    if "/root/.axon_site" not in sys.path:
        sys.path.insert(0, "/root/.axon_site")
    from trn_agent_boot.trn_boot import _ntff_profile_via_ctypes

    hook = _ntff_profile_via_ctypes("/opt/axon/libaxon_pjrt.so")
    mod = types.ModuleType("antenv.axon_hooks")
    _state = {"hook": hook}
    mod.get_axon_ntff_profile_hook = lambda: _state["hook"]
    mod.set_axon_ntff_profile_hook = lambda h: _state.__setitem__("hook", h)
    sys.modules["antenv.axon_hooks"] = mod
    import antenv

    antenv.axon_hooks = mod


def run_kernel(x, W, trace=False):
    """Returns (output [256,10,16,1] f32, BassKernelResults)."""
    if trace:
        _ensure_ntff_hook()
    nc = _get_program()
    in_maps = _prep_inputs(x, W)
    bkr = run_bass_kernel_spmd(nc, in_maps, list(range(NCORES)), trace=trace)
    v = np.concatenate([bkr.results[c]["vout"] for c in range(NCORES)], axis=0)
    return v[..., None].astype(np.float32), bkr


def kernel(x, W):
    try:
        out, _ = run_kernel(x, W, trace=False)
    except Exception:
        # one retry: a previous aborted run can leave a core wedged once
        out, _ = run_kernel(x, W, trace=False)
    return out


# revision 23
# speedup vs baseline: 1.0320x; 1.0320x over previous
"""DGL DigitCapsuleLayer (dynamic routing, 3 iters) on 8 trn2 NeuronCores.

Strategy: data-parallel over batch B=256 (32 per core), W replicated.
u_hat ([1152,10,256,16] = 189MB) is NEVER materialized; both routing
contractions are computed as fused matmul pipelines from x and W:

  s[j,b,d]  = sum_{i,c} (c[i,j] * W[i,j,d,c]) * x[b,c,i]      (PE, 72 matmuls)
  t[i,j]    = sum_{c,d} W[i,j,d,c] * y[j,d,c,i],
  y[j,d,c,i]= sum_b v[j,b,d] * x[b,c,i]                        (PE + DVE reduce)

All big tensors keep i on the partition axis (i = k*128 + p, k in 0..8), so
softmax / the c*W product / the (c,d)-reduction are pure free-dim ops --
no cross-partition transpose anywhere.  b_ij updates need 2 AllReduces of
the [1152,10] agreement matrix (iter-1 softmax is uniform; the post-iter-3
update is dead code).
"""

import sys

for _p in ("/opt/trn_rl_repo",):
    if _p not in sys.path:
        sys.path.insert(0, _p)

import numpy as np

import concourse.bacc as bacc
import concourse.bass as bass
import concourse.mybir as mybir
import concourse.tile as tile
from concourse.bass_utils import run_bass_kernel_spmd

F32 = mybir.dt.float32
BF16 = mybir.dt.bfloat16
AX = mybir.AxisListType
ALU = mybir.AluOpType
ACTF = mybir.ActivationFunctionType

NCORES = 8
B = 256
BL = B // NCORES          # 32 batch rows per core
I = 1152                  # input capsules
K = I // 128              # 9 i-chunks of 128
P = 128
C = 8                     # in_dim
J = 10                    # out capsules
D = 16                    # out_dim
JD = J * D                # 160
KCJD = K * C * J * D      # 11520

_PROG = None


def _build_program():
    nc = bacc.Bacc(
        "TRN2",
        target_bir_lowering=False,
        debug=False,
        num_devices=NCORES,
    )

    G = K * C // 4            # 18 rounds of 4 PE tiles

    xs_d = nc.declare_dram_parameter("xs", [P, C, K, BL], F32, isOutput=False)
    xt_d = nc.declare_dram_parameter("xt", [P, G, P], F32, isOutput=False)
    wp_d = nc.declare_dram_parameter("wp", [P, C, K, J, D], F32, isOutput=False)
    eye_d = nc.declare_dram_parameter("eye", [P, P], F32, isOutput=False)
    vout_d = nc.declare_dram_parameter("vout", [BL, J, D], F32, isOutput=True)

    with tile.TileContext(nc) as tc:
        with (
            tc.tile_pool(name="sb", bufs=1) as sb,
            tc.tile_pool(name="ps", bufs=2, space="PSUM") as ps,
            tc.tile_pool(name="dram", bufs=1, space="DRAM") as dram,
        ):
            # ---- persistent SBUF tensors ----
            w1h = sb.tile([P, C, K, J, D], BF16)          # W, i-partitioned
            x1h = sb.tile([P, C, K, BL], BF16)            # lhsT for s-matmuls
            x2h = sb.tile([P, G, P], BF16)                # lhsT strips for T1
            cw = sb.tile([P, C, K, J, D], BF16)           # c[i,j] * W
            yh = sb.tile([P, C, K, J, D], BF16)           # y, i-partitioned
            ph = sb.tile([P, C, K, J, D], BF16, tag="bigscratch")  # W .* y
            eye_sb = sb.tile([P, P], F32)                 # col-tile combiner
            sfull = sb.tile([P, JD], F32)                 # col-tile partials
            chd = sb.tile([P, K, J, D], BF16)             # c bcast over d
            w1h_f = w1h.rearrange("p c k j d -> p (c k j d)")
            yh_f = yh.rearrange("p c k j d -> p (c k j d)")
            ph_f = ph.rearrange("p c k j d -> p (c k j d)")

            # ---- load + cast (fp32 DMA staging -> bf16) ----
            # x / eye first (small; unblocks it1's s-phase early), then the W
            # quarters. Disjoint staging ranges: dynamic HWDGE DMAs only
            # support one sync-wait slot, so no buffer reuse here.
            nc.sync.dma_start(eye_sb[:, :], eye_d[:, :])

            x1_stage = sb.tile([P, K * C * BL], F32)
            nc.sync.dma_start(x1_stage[:, :], xs_d.rearrange("p c k b -> p (c k b)"))
            nc.scalar.copy(x1h.rearrange("p c k b -> p (c k b)"), x1_stage[:, :])

            w_stage = sb.tile([P, KCJD], F32, tag="bigscratch")
            NQ = 8
            QW = KCJD // NQ
            for q in range(NQ):
                nc.sync.dma_start(
                    w_stage[:, q * QW:(q + 1) * QW],
                    wp_d.rearrange("p c k j d -> p (c k j d)")[:, q * QW:(q + 1) * QW],
                )
                lo, mid, hi = q * QW, q * QW + (QW * 5) // 8, (q + 1) * QW
                nc.vector.tensor_copy(w1h_f[:, lo:mid], w_stage[:, lo:mid])
                nc.scalar.copy(w1h_f[:, mid:hi], w_stage[:, mid:hi])

            x2_stage = sb.tile([P, G * P], F32)
            nc.sync.dma_start(x2_stage[:, :], xt_d.rearrange("p g q -> p (g q)"))
            nc.scalar.copy(x2h.rearrange("p g q -> p (g q)"), x2_stage[:, :])

            # ---- DRAM bounce buffers for the 2 AllReduces ----
            t_b1 = dram.tile([P, K * J], BF16)
            t_b2 = dram.tile([P, K * J], BF16)
            t_ar1 = dram.tile([P, K * J], BF16)
            t_ar2 = dram.tile([P, K * J], BF16)

            # ---- small SBUF working tensors ----
            bsum = sb.tile([P, K, J], F32)     # running sum of AllReduced t

            def softmax_to_cw():
                """c = softmax(bsum/256, axis=j); cw = c * W (bf16)."""
                # no max-subtraction: |t_sum|/256 is bounded by ~3, exp is
                # safe in fp32 and softmax is shift-invariant.
                ex = sb.tile([P, K, J], F32, tag="smx_ex")
                nc.scalar.activation(ex[:, :, :], bsum[:, :, :], ACTF.Exp,
                                     scale=1.0 / 256.0)
                ssum = sb.tile([P, K], F32, tag="smx_s")
                nc.vector.tensor_reduce(ssum[:, :], ex[:, :, :], axis=AX.X,
                                        op=ALU.add)
                rs = sb.tile([P, K], F32, tag="smx_r")
                nc.vector.reciprocal(rs[:, :], ssum[:, :])
                ch = sb.tile([P, K, J], F32, tag="smx_c")
                nc.vector.tensor_mul(
                    ch[:, :, :], ex[:, :, :],
                    rs.unsqueeze(2).broadcast_to([P, K, J]),
                )
                # chd[p,k,j,d] = c (materialized over d so the 8 per-c
                # products below hit the DVE 2x bf16 mode: step-1 innermost)
                nc.scalar.copy(
                    chd[:, :, :, :],
                    ch.unsqueeze(3).broadcast_to([P, K, J, D]),
                )
                for c in range(C):
                    nc.vector.tensor_mul(
                        cw[:, c, :, :, :].rearrange("p k j d -> p k (j d)"),
                        w1h[:, c, :, :, :].rearrange("p k j d -> p k (j d)"),
                        chd.rearrange("p k j d -> p k (j d)"),
                    )

            def s_phase(rhs_tile):
                """72 matmuls on 4 concurrent PE col-tiles, partials combined
                (and replicated x4 for T1's row tiles) by the eye matmul."""
                scol = ps.tile([P, 4, 512], F32, tag="ps4", name="scol")
                for kk in range(K * C):
                    c, k = divmod(kk, K)
                    ct, g = kk % 4, kk // 4
                    nc.tensor.matmul(
                        scol[32 * ct:32 * ct + 32, ct, 0:JD],
                        lhsT=x1h[:, c, k, :],
                        rhs=rhs_tile[:, c, k, :, :].rearrange("p j d -> p (j d)"),
                        start=(g == 0),
                        stop=(g == G - 1),
                        tile_position=(0, 32 * ct),
                    )
                for ct in range(4):
                    eng = nc.scalar if ct % 2 == 0 else nc.vector
                    if ct % 2 == 0:
                        nc.scalar.copy(sfull[32 * ct:32 * ct + 32, :],
                                       scol[32 * ct:32 * ct + 32, ct, 0:JD])
                    else:
                        nc.vector.tensor_copy(sfull[32 * ct:32 * ct + 32, :],
                                              scol[32 * ct:32 * ct + 32, ct, 0:JD])
                s2 = ps.tile([P, 4, 512], F32, tag="ps4", name="s2")
                nc.tensor.matmul(
                    s2[:, 0, 0:JD],
                    lhsT=eye_sb[:, :],
                    rhs=sfull[:, :],
                    start=True,
                    stop=True,
                )
                return s2[:, 0, 0:JD]

            def squash(s_ps, uniform, out_tile):
                """out_tile[p, j, d] = squash(scale * s); s is x4-replicated
                across the partition row-groups, so out is too.  Uses
                squash(s) = s * sqrt(|s|^2) / (1 + |s|^2)."""
                scale = 0.1 if uniform else 1.0
                s_sb = sb.tile([P, J, D], F32, tag="sq_s")
                nc.scalar.activation(s_sb.rearrange("p j d -> p (j d)"), s_ps,
                                     ACTF.Copy, scale=scale)
                ssq = sb.tile([P, J, D], F32, tag="sq_ssq")
                nc.vector.tensor_mul(ssq[:, :, :], s_sb[:, :, :], s_sb[:, :, :])
                sq = sb.tile([P, J], F32, tag="sq_sq")
                nc.vector.tensor_reduce(sq[:, :], ssq[:, :, :], axis=AX.X,
                                        op=ALU.add)
                rt = sb.tile([P, J], F32, tag="sq_rt")
                nc.scalar.activation(rt[:, :], sq[:, :], ACTF.Sqrt)
                sp1 = sb.tile([P, J], F32, tag="sq_sp1")
                nc.vector.tensor_scalar_add(sp1[:, :], sq[:, :], 1.0)
                fi = sb.tile([P, J], F32, tag="sq_fi")
                nc.vector.reciprocal(fi[:, :], sp1[:, :])
                g = sb.tile([P, J], F32, tag="sq_g")
                nc.vector.tensor_mul(g[:, :], rt[:, :], fi[:, :])
                nc.vector.tensor_mul(
                    out_tile[:, :, :],
                    s_sb[:, :, :],
                    g.unsqueeze(2).broadcast_to([P, J, D]),
                )

            def t_phase(vh4, t_bounce, t_arout, it):
                """T1 on 4 concurrent PE row-tiles; T2 = mul + bf16 add-tree;
                AllReduce of the [1152,10] agreement matrix."""
                vh4_f = vh4.rearrange("p j d -> p (j d)")
                QC = KCJD // 4          # one c-pair (quarter) of ph
                HC = QC // 2            # 1440: fully c-reduced slice size
                tQ = sb.tile([P, 4 * HC], BF16, tag="treeQ", name=f"tQ{it}")
                for g in range(G):
                    yp = ps.tile([P, 4, 512], F32, tag="ps4", name=f"yp{it}_{g}")
                    for r in range(4):
                        nc.tensor.matmul(
                            yp[:, r, 0:JD],
                            lhsT=x2h[32 * r:32 * r + 32, g, :],
                            rhs=vh4_f[32 * r:32 * r + 32, :],
                            start=True,
                            stop=True,
                            tile_position=(32 * r, 0),
                        )
                    rng = slice(g * 4 * JD, (g + 1) * 4 * JD)
                    nc.scalar.copy(yh_f[:, rng], yp[:, :, 0:JD])
                    # Quarter-wise W .* y + intra-quarter c-pair reduce on DVE,
                    # pipelined behind the (ACT-only) PSUM evacuation.
                    if g in (4, 8, 13):
                        qi = {4: 0, 8: 1, 13: 2}[g]
                        qs = slice(qi * QC, (qi + 1) * QC)
                        nc.vector.tensor_mul(ph_f[:, qs], w1h_f[:, qs],
                                             yh_f[:, qs])
                        nc.vector.tensor_add(
                            tQ[:, qi * HC:(qi + 1) * HC],
                            ph_f[:, qi * QC:qi * QC + HC],
                            ph_f[:, qi * QC + HC:(qi + 1) * QC])
                qs = slice(3 * QC, 4 * QC)
                nc.vector.tensor_mul(ph_f[:, qs], w1h_f[:, qs], yh_f[:, qs])
                nc.vector.tensor_add(tQ[:, 3 * HC:4 * HC],
                                     ph_f[:, 3 * QC:3 * QC + HC],
                                     ph_f[:, 3 * QC + HC:4 * QC])
                u01 = sb.tile([P, HC], BF16, tag="treeU01")
                nc.vector.tensor_add(u01[:, :], tQ[:, 0:HC], tQ[:, HC:2 * HC])
                u23 = sb.tile([P, HC], BF16, tag="treeU23")
                nc.vector.tensor_add(u23[:, :], tQ[:, 2 * HC:3 * HC],
                                     tQ[:, 3 * HC:4 * HC])
                tC = sb.tile([P, K, J, D], BF16, tag="treeC")
                nc.vector.tensor_add(
                    tC.rearrange("p k j d -> p (k j d)"), u01[:, :], u23[:, :])
                tD = sb.tile([P, K, J, 8], BF16, tag="treeD")
                nc.vector.tensor_add(tD[:, :, :, :], tC[:, :, :, 0:8],
                                     tC[:, :, :, 8:16])
                tE = sb.tile([P, K, J, 4], BF16, tag="treeE")
                nc.vector.tensor_add(tE[:, :, :, :], tD[:, :, :, 0:4],
                                     tD[:, :, :, 4:8])
                tF = sb.tile([P, K, J, 2], BF16, tag="treeF")
                nc.vector.tensor_add(tF[:, :, :, :], tE[:, :, :, 0:2],
                                     tE[:, :, :, 2:4])
                tloc = sb.tile([P, K, J], BF16, name=f"tloc{it}")
                nc.vector.tensor_add(tloc[:, :, :], tF[:, :, :, 0],
                                     tF[:, :, :, 1])
                nc.gpsimd.dma_start(
                    t_bounce[:, :], tloc.rearrange("p k j -> p (k j)")
                )
                nc.gpsimd.collective_compute(
                    "AllReduce",
                    ALU.add,
                    replica_groups=[list(range(NCORES))],
                    ins=[t_bounce[:, :]],
                    outs=[t_arout[:, :]],
                )

            def ag_sum(ag_out, target, it):
                """Load the AllReduced [128, 90] sum into target (f32)."""
                st = sb.tile([P, K * J], BF16, tag=f"agst{it}")
                nc.gpsimd.dma_start(st[:, :], ag_out[:, :])
                nc.vector.tensor_copy(
                    target.rearrange("p k j -> p (k j)"), st[:, :])

            # ================= iteration 1 (c uniform = 1/10) =================
            v1 = sb.tile([P, J, D], BF16)
            s_ps = s_phase(w1h)
            squash(s_ps, True, v1)
            t_phase(v1, t_b1, t_ar1, 1)
            ag_sum(t_ar1, bsum, 1)

            # ================= iteration 2 =================
            softmax_to_cw()
            v2 = sb.tile([P, J, D], BF16)
            s_ps = s_phase(cw)
            squash(s_ps, False, v2)
            t_phase(v2, t_b2, t_ar2, 2)
            tsum2 = sb.tile([P, K, J], F32)
            ag_sum(t_ar2, tsum2, 2)
            nc.vector.tensor_add(bsum[:, :, :], bsum[:, :, :], tsum2[:, :, :])

            # ================= iteration 3 (no b update needed) ===============
            softmax_to_cw()
            v3 = sb.tile([P, J, D], F32)
            s_ps = s_phase(cw)
            squash(s_ps, False, v3)
            nc.gpsimd.dma_start(vout_d[:, :, :], v3[0:BL, :, :])

    nc.compile()
    return nc


def _get_program():
    global _PROG
    if _PROG is None:
        _PROG = _build_program()
    return _PROG


def _prep_inputs(x, W):
    """Host-side shard + layout permutation (pure reshapes, fp32 preserved)."""
    x = np.asarray(x, dtype=np.float32)
    W = np.asarray(W, dtype=np.float32)
    # wp[p, c, k, j, d] = W[k*128+p, j, d, c]  (c-major chunk order so the
    # T2 c-reduction is a flat halves tree)
    wp = np.ascontiguousarray(
        W.reshape(K, P, J, D, C).transpose(1, 4, 0, 2, 3))
    # eye[p, m] = 1 iff p == m (mod 32): the col-tile partial combiner,
    # which also replicates s across the 4 row-groups for T1's row tiles.
    eye = np.tile(np.eye(32, dtype=np.float32), (4, 4))
    in_maps = []
    for core in range(NCORES):
        xsh = x[core * BL:(core + 1) * BL]          # [BL, C, I]
        # xs[p, c, k, b] = xsh[b, c, k*128+p]
        xs = np.ascontiguousarray(
            xsh.transpose(2, 1, 0).reshape(K, P, C, BL).transpose(1, 2, 0, 3))
        # xt[32*(kk%4)+b, kk//4, p] = xsh[b, c, k*128+p] for kk=(c*K+k):
        # lhsT strips for the 4 concurrent T1 row-tiles.
        xt = np.empty((P, K * C // 4, P), dtype=np.float32)
        for kk in range(K * C):
            c, k = divmod(kk, K)
            r, g = kk % 4, kk // 4
            xt[32 * r:32 * r + 32, g, :] = xsh[:, c, k * P:(k + 1) * P]
        in_maps.append({"xs": xs, "xt": xt, "wp": wp, "eye": eye})
    return in_maps


def _ensure_ntff_hook():
    """Register the axon NTFF profile hook if the image's antenv lacks it."""
    import types

    try:
        from antenv.axon_hooks import get_axon_ntff_profile_hook  # noqa: F401
        return
    except ImportError:
        pass
    if "/root/.axon_site" not in sys.path:
        sys.path.insert(0, "/root/.axon_site")
    from trn_agent_boot.trn_boot import _ntff_profile_via_ctypes

    hook = _ntff_profile_via_ctypes("/opt/axon/libaxon_pjrt.so")
    mod = types.ModuleType("antenv.axon_hooks")
    _state = {"hook": hook}
    mod.get_axon_ntff_profile_hook = lambda: _state["hook"]
    mod.set_axon_ntff_profile_hook = lambda h: _state.__setitem__("hook", h)
    sys.modules["antenv.axon_hooks"] = mod
    import antenv

    antenv.axon_hooks = mod


def run_kernel(x, W, trace=False):
    """Returns (output [256,10,16,1] f32, BassKernelResults)."""
    if trace:
        _ensure_ntff_hook()
    nc = _get_program()
    in_maps = _prep_inputs(x, W)
    bkr = run_bass_kernel_spmd(nc, in_maps, list(range(NCORES)), trace=trace)
    v = np.concatenate([bkr.results[c]["vout"] for c in range(NCORES)], axis=0)
    return v[..., None].astype(np.float32), bkr


def kernel(x, W):
    try:
        out, _ = run_kernel(x, W, trace=False)
    except Exception:
        # one retry: a previous aborted run can leave a core wedged once
        out, _ = run_kernel(x, W, trace=False)
    return out


# revision 24
# speedup vs baseline: 1.1550x; 1.1192x over previous
"""DGL DigitCapsuleLayer (dynamic routing, 3 iters) on 8 trn2 NeuronCores.

Strategy: data-parallel over batch B=256 (32 per core), W replicated.
u_hat ([1152,10,256,16] = 189MB) is NEVER materialized; both routing
contractions are computed as fused matmul pipelines from x and W:

  s[j,b,d]  = sum_{i,c} (c[i,j] * W[i,j,d,c]) * x[b,c,i]      (PE, 72 matmuls)
  t[i,j]    = sum_{c,d} W[i,j,d,c] * y[j,d,c,i],
  y[j,d,c,i]= sum_b v[j,b,d] * x[b,c,i]                        (PE + DVE reduce)

All big tensors keep i on the partition axis (i = k*128 + p, k in 0..8), so
softmax / the c*W product / the (c,d)-reduction are pure free-dim ops --
no cross-partition transpose anywhere.  b_ij updates need 2 AllReduces of
the [1152,10] agreement matrix (iter-1 softmax is uniform; the post-iter-3
update is dead code).
"""

import sys

for _p in ("/opt/trn_rl_repo",):
    if _p not in sys.path:
        sys.path.insert(0, _p)

import numpy as np

import concourse.bacc as bacc
import concourse.bass as bass
import concourse.mybir as mybir
import concourse.tile as tile
from concourse.bass_utils import run_bass_kernel_spmd

F32 = mybir.dt.float32
BF16 = mybir.dt.bfloat16
AX = mybir.AxisListType
ALU = mybir.AluOpType
ACTF = mybir.ActivationFunctionType

NCORES = 8
B = 256
BL = B // NCORES          # 32 batch rows per core
I = 1152                  # input capsules
K = I // 128              # 9 i-chunks of 128
P = 128
C = 8                     # in_dim
J = 10                    # out capsules
D = 16                    # out_dim
JD = J * D                # 160
KCJD = K * C * J * D      # 11520

_PROG = None


def _build_program():
    nc = bacc.Bacc(
        "TRN2",
        target_bir_lowering=False,
        debug=False,
        num_devices=NCORES,
    )

    G = K * C // 4            # 18 rounds of 4 PE tiles

    xs_d = nc.declare_dram_parameter("xs", [P, C, K, BL], F32, isOutput=False)
    xt_d = nc.declare_dram_parameter("xt", [P, G, P], F32, isOutput=False)
    wp_d = nc.declare_dram_parameter("wp", [P, C, K, J, D], F32, isOutput=False)
    eye_d = nc.declare_dram_parameter("eye", [P, P], BF16, isOutput=False)
    vout_d = nc.declare_dram_parameter("vout", [BL, J, D], F32, isOutput=True)

    with tile.TileContext(nc) as tc:
        with (
            tc.tile_pool(name="sb", bufs=1) as sb,
            tc.tile_pool(name="ps", bufs=2, space="PSUM") as ps,
            tc.tile_pool(name="dram", bufs=1, space="DRAM") as dram,
        ):
            # ---- persistent SBUF tensors ----
            w1h = sb.tile([P, C, K, J, D], BF16)          # W, i-partitioned
            x1h = sb.tile([P, C, K, BL], BF16)            # lhsT for s-matmuls
            x2h = sb.tile([P, G, P], BF16)                # lhsT strips for T1
            cw = sb.tile([P, C, K, J, D], BF16)           # c[i,j] * W
            yh = sb.tile([P, C, K, J, D], BF16)           # y, i-partitioned
            ph = sb.tile([P, C, K, J, D], BF16, tag="bigscratch")  # W .* y
            eye_sb = sb.tile([P, P], BF16)                # col-tile combiner
            sfull = sb.tile([P, JD], BF16)                # col-tile partials
            chd = sb.tile([P, K, J, D], BF16)             # c bcast over d
            w1h_f = w1h.rearrange("p c k j d -> p (c k j d)")
            yh_f = yh.rearrange("p c k j d -> p (c k j d)")
            ph_f = ph.rearrange("p c k j d -> p (c k j d)")

            # ---- load + cast (fp32 DMA staging -> bf16) ----
            # x / eye first (small; unblocks it1's s-phase early), then the W
            # quarters. Disjoint staging ranges: dynamic HWDGE DMAs only
            # support one sync-wait slot, so no buffer reuse here.
            nc.sync.dma_start(eye_sb[:, :], eye_d[:, :])

            x1_stage = sb.tile([P, K * C * BL], F32)
            nc.sync.dma_start(x1_stage[:, :], xs_d.rearrange("p c k b -> p (c k b)"))
            nc.scalar.copy(x1h.rearrange("p c k b -> p (c k b)"), x1_stage[:, :])

            w_stage = sb.tile([P, KCJD], F32, tag="bigscratch")
            NQ = 8
            QW = KCJD // NQ
            for q in range(NQ):
                nc.sync.dma_start(
                    w_stage[:, q * QW:(q + 1) * QW],
                    wp_d.rearrange("p c k j d -> p (c k j d)")[:, q * QW:(q + 1) * QW],
                )
                lo, mid, hi = q * QW, q * QW + (QW * 5) // 8, (q + 1) * QW
                nc.vector.tensor_copy(w1h_f[:, lo:mid], w_stage[:, lo:mid])
                nc.scalar.copy(w1h_f[:, mid:hi], w_stage[:, mid:hi])

            x2_stage = sb.tile([P, G * P], F32)
            nc.sync.dma_start(x2_stage[:, :], xt_d.rearrange("p g q -> p (g q)"))
            nc.scalar.copy(x2h.rearrange("p g q -> p (g q)"), x2_stage[:, :])

            # ---- DRAM bounce buffers for the 2 AllReduces ----
            t_b1 = dram.tile([P, K * J], BF16)
            t_b2 = dram.tile([P, K * J], BF16)
            t_ar1 = dram.tile([P, K * J], BF16)
            t_ar2 = dram.tile([P, K * J], BF16)

            # ---- small SBUF working tensors ----
            bsum = sb.tile([P, K, J], F32)     # running sum of AllReduced t

            def softmax_to_cw():
                """c = softmax(bsum/256, axis=j); cw = c * W (bf16)."""
                # no max-subtraction: |t_sum|/256 is bounded by ~3, exp is
                # safe in fp32 and softmax is shift-invariant.
                ex = sb.tile([P, K, J], F32, tag="smx_ex")
                nc.scalar.activation(ex[:, :, :], bsum[:, :, :], ACTF.Exp,
                                     scale=1.0 / 256.0)
                ssum = sb.tile([P, K], F32, tag="smx_s")
                nc.vector.tensor_reduce(ssum[:, :], ex[:, :, :], axis=AX.X,
                                        op=ALU.add)
                rs = sb.tile([P, K], F32, tag="smx_r")
                nc.vector.reciprocal(rs[:, :], ssum[:, :])
                ch = sb.tile([P, K, J], F32, tag="smx_c")
                nc.vector.tensor_mul(
                    ch[:, :, :], ex[:, :, :],
                    rs.unsqueeze(2).broadcast_to([P, K, J]),
                )
                # chd[p,k,j,d] = c (materialized over d so the 8 per-c
                # products below hit the DVE 2x bf16 mode: step-1 innermost)
                nc.scalar.copy(
                    chd[:, :, :, :],
                    ch.unsqueeze(3).broadcast_to([P, K, J, D]),
                )
                for c in range(C):
                    nc.vector.tensor_mul(
                        cw[:, c, :, :, :].rearrange("p k j d -> p k (j d)"),
                        w1h[:, c, :, :, :].rearrange("p k j d -> p k (j d)"),
                        chd.rearrange("p k j d -> p k (j d)"),
                    )

            def s_phase(rhs_tile):
                """72 matmuls on 4 concurrent PE col-tiles, partials combined
                (and replicated x4 for T1's row tiles) by the eye matmul."""
                scol = ps.tile([P, 4, 512], F32, tag="ps4", name="scol")
                for kk in range(K * C):
                    c, k = divmod(kk, K)
                    ct, g = kk % 4, kk // 4
                    nc.tensor.matmul(
                        scol[32 * ct:32 * ct + 32, ct, 0:JD],
                        lhsT=x1h[:, c, k, :],
                        rhs=rhs_tile[:, c, k, :, :].rearrange("p j d -> p (j d)"),
                        start=(g == 0),
                        stop=(g == G - 1),
                        tile_position=(0, 32 * ct),
                    )
                for ct in range(4):
                    eng = nc.scalar if ct % 2 == 0 else nc.vector
                    if ct % 2 == 0:
                        nc.scalar.copy(sfull[32 * ct:32 * ct + 32, :],
                                       scol[32 * ct:32 * ct + 32, ct, 0:JD])
                    else:
                        nc.vector.tensor_copy(sfull[32 * ct:32 * ct + 32, :],
                                              scol[32 * ct:32 * ct + 32, ct, 0:JD])
                s2 = ps.tile([P, 4, 512], F32, tag="ps4", name="s2")
                nc.tensor.matmul(
                    s2[:, 0, 0:JD],
                    lhsT=eye_sb[:, :],
                    rhs=sfull[:, :],
                    start=True,
                    stop=True,
                )
                return s2[:, 0, 0:JD]

            def squash(s_ps, uniform, out_tile):
                """out_tile[p, j, d] = squash(scale * s); s is x4-replicated
                across the partition row-groups, so out is too.  Uses
                squash(s) = s * sqrt(|s|^2) / (1 + |s|^2)."""
                scale = 0.1 if uniform else 1.0
                s3 = s_ps.rearrange("b (j d) -> b j d", d=D)
                ssq = sb.tile([P, J, D], F32, tag="sq_ssq")
                nc.scalar.activation(ssq[:, :, :], s3, ACTF.Square, scale=scale)
                sq = sb.tile([P, J], F32, tag="sq_sq")
                nc.vector.tensor_reduce(sq[:, :], ssq[:, :, :], axis=AX.X,
                                        op=ALU.add)
                rt = sb.tile([P, J], F32, tag="sq_rt")
                nc.scalar.activation(rt[:, :], sq[:, :], ACTF.Sqrt)
                sp1 = sb.tile([P, J], F32, tag="sq_sp1")
                nc.vector.tensor_scalar_add(sp1[:, :], sq[:, :], 1.0)
                fi = sb.tile([P, J], F32, tag="sq_fi")
                nc.vector.reciprocal(fi[:, :], sp1[:, :])
                g = sb.tile([P, J], F32, tag="sq_g")
                nc.vector.tensor_mul(g[:, :], rt[:, :], fi[:, :])
                nc.vector.scalar_tensor_tensor(
                    out_tile[:, :, :],
                    in0=s3,
                    scalar=scale,
                    in1=g.unsqueeze(2).broadcast_to([P, J, D]),
                    op0=ALU.mult,
                    op1=ALU.mult,
                )

            def t_phase(vh4, t_bounce, t_arout, it):
                """T1 on 4 concurrent PE row-tiles; T2 = mul + bf16 add-tree;
                AllReduce of the [1152,10] agreement matrix."""
                vh4_f = vh4.rearrange("p j d -> p (j d)")
                QC = KCJD // 4          # one c-pair (quarter) of ph
                HC = QC // 2            # 1440: fully c-reduced slice size
                tQ = sb.tile([P, 4 * HC], BF16, tag="treeQ", name=f"tQ{it}")
                for g in range(G):
                    yp = ps.tile([P, 4, 512], F32, tag="ps4", name=f"yp{it}_{g}")
                    for r in range(4):
                        nc.tensor.matmul(
                            yp[:, r, 0:JD],
                            lhsT=x2h[32 * r:32 * r + 32, g, :],
                            rhs=vh4_f[32 * r:32 * r + 32, :],
                            start=True,
                            stop=True,
                            tile_position=(32 * r, 0),
                        )
                    rng = slice(g * 4 * JD, (g + 1) * 4 * JD)
                    nc.scalar.copy(yh_f[:, rng], yp[:, :, 0:JD])
                    # Quarter-wise W .* y + intra-quarter c-pair reduce on DVE,
                    # pipelined behind the (ACT-only) PSUM evacuation.
                    if g in (4, 8, 13):
                        qi = {4: 0, 8: 1, 13: 2}[g]
                        qs = slice(qi * QC, (qi + 1) * QC)
                        nc.vector.tensor_mul(ph_f[:, qs], w1h_f[:, qs],
                                             yh_f[:, qs])
                        nc.vector.tensor_add(
                            tQ[:, qi * HC:(qi + 1) * HC],
                            ph_f[:, qi * QC:qi * QC + HC],
                            ph_f[:, qi * QC + HC:(qi + 1) * QC])
                qs = slice(3 * QC, 4 * QC)
                nc.vector.tensor_mul(ph_f[:, qs], w1h_f[:, qs], yh_f[:, qs])
                nc.vector.tensor_add(tQ[:, 3 * HC:4 * HC],
                                     ph_f[:, 3 * QC:3 * QC + HC],
                                     ph_f[:, 3 * QC + HC:4 * QC])
                u01 = sb.tile([P, HC], BF16, tag="treeU01")
                nc.vector.tensor_add(u01[:, :], tQ[:, 0:HC], tQ[:, HC:2 * HC])
                u23 = sb.tile([P, HC], BF16, tag="treeU23")
                nc.vector.tensor_add(u23[:, :], tQ[:, 2 * HC:3 * HC],
                                     tQ[:, 3 * HC:4 * HC])
                tC = sb.tile([P, K, J, D], BF16, tag="treeC")
                nc.vector.tensor_add(
                    tC.rearrange("p k j d -> p (k j d)"), u01[:, :], u23[:, :])
                tD = sb.tile([P, K, J, 8], BF16, tag="treeD")
                nc.vector.tensor_add(tD[:, :, :, :], tC[:, :, :, 0:8],
                                     tC[:, :, :, 8:16])
                tE = sb.tile([P, K, J, 4], BF16, tag="treeE")
                nc.vector.tensor_add(tE[:, :, :, :], tD[:, :, :, 0:4],
                                     tD[:, :, :, 4:8])
                tF = sb.tile([P, K, J, 2], BF16, tag="treeF")
                nc.vector.tensor_add(tF[:, :, :, :], tE[:, :, :, 0:2],
                                     tE[:, :, :, 2:4])
                tloc = sb.tile([P, K, J], BF16, name=f"tloc{it}")
                nc.vector.tensor_add(tloc[:, :, :], tF[:, :, :, 0],
                                     tF[:, :, :, 1])
                nc.gpsimd.dma_start(
                    t_bounce[:, :], tloc.rearrange("p k j -> p (k j)")
                )
                nc.gpsimd.collective_compute(
                    "AllReduce",
                    ALU.add,
                    replica_groups=[list(range(NCORES))],
                    ins=[t_bounce[:, :]],
                    outs=[t_arout[:, :]],
                )

            def ag_sum(ag_out, target, it):
                """Load the AllReduced [128, 90] sum into target (f32)."""
                st = sb.tile([P, K * J], BF16, tag=f"agst{it}")
                nc.gpsimd.dma_start(st[:, :], ag_out[:, :])
                nc.vector.tensor_copy(
                    target.rearrange("p k j -> p (k j)"), st[:, :])

            # ================= iteration 1 (c uniform = 1/10) =================
            v1 = sb.tile([P, J, D], BF16)
            s_ps = s_phase(w1h)
            squash(s_ps, True, v1)
            t_phase(v1, t_b1, t_ar1, 1)
            ag_sum(t_ar1, bsum, 1)

            # ================= iteration 2 =================
            softmax_to_cw()
            v2 = sb.tile([P, J, D], BF16)
            s_ps = s_phase(cw)
            squash(s_ps, False, v2)
            t_phase(v2, t_b2, t_ar2, 2)
            tsum2 = sb.tile([P, K, J], F32)
            ag_sum(t_ar2, tsum2, 2)
            nc.vector.tensor_add(bsum[:, :, :], bsum[:, :, :], tsum2[:, :, :])

            # ================= iteration 3 (no b update needed) ===============
            softmax_to_cw()
            v3 = sb.tile([P, J, D], F32)
            s_ps = s_phase(cw)
            squash(s_ps, False, v3)
            nc.gpsimd.dma_start(vout_d[:, :, :], v3[0:BL, :, :])

    nc.compile()
    return nc


def _get_program():
    global _PROG
    if _PROG is None:
        _PROG = _build_program()
    return _PROG


def _prep_inputs(x, W):
    """Host-side shard + layout permutation (pure reshapes, fp32 preserved)."""
    x = np.asarray(x, dtype=np.float32)
    W = np.asarray(W, dtype=np.float32)
    # wp[p, c, k, j, d] = W[k*128+p, j, d, c]  (c-major chunk order so the
    # T2 c-reduction is a flat halves tree)
    wp = np.ascontiguousarray(
        W.reshape(K, P, J, D, C).transpose(1, 4, 0, 2, 3))
    # eye[p, m] = 1 iff p == m (mod 32): the col-tile partial combiner,
    # which also replicates s across the 4 row-groups for T1's row tiles.
    from ml_dtypes import bfloat16
    eye = np.tile(np.eye(32, dtype=np.float32), (4, 4)).astype(bfloat16)
    in_maps = []
    for core in range(NCORES):
        xsh = x[core * BL:(core + 1) * BL]          # [BL, C, I]
        # xs[p, c, k, b] = xsh[b, c, k*128+p]
        xs = np.ascontiguousarray(
            xsh.transpose(2, 1, 0).reshape(K, P, C, BL).transpose(1, 2, 0, 3))
        # xt[32*(kk%4)+b, kk//4, p] = xsh[b, c, k*128+p] for kk=(c*K+k):
        # lhsT strips for the 4 concurrent T1 row-tiles.
        xt = np.empty((P, K * C // 4, P), dtype=np.float32)
        for kk in range(K * C):
            c, k = divmod(kk, K)
            r, g = kk % 4, kk // 4
            xt[32 * r:32 * r + 32, g, :] = xsh[:, c, k * P:(k + 1) * P]
        in_maps.append({"xs": xs, "xt": xt, "wp": wp, "eye": eye})
    return in_maps


def _ensure_ntff_hook():
    """Register the axon NTFF profile hook if the image's antenv lacks it."""
    import types

    try:
        from antenv.axon_hooks import get_axon_ntff_profile_hook  # noqa: F401
        return
    except ImportError:
        pass
    if "/root/.axon_site" not in sys.path:
        sys.path.insert(0, "/root/.axon_site")
    from trn_agent_boot.trn_boot import _ntff_profile_via_ctypes

    hook = _ntff_profile_via_ctypes("/opt/axon/libaxon_pjrt.so")
    mod = types.ModuleType("antenv.axon_hooks")
    _state = {"hook": hook}
    mod.get_axon_ntff_profile_hook = lambda: _state["hook"]
    mod.set_axon_ntff_profile_hook = lambda h: _state.__setitem__("hook", h)
    sys.modules["antenv.axon_hooks"] = mod
    import antenv

    antenv.axon_hooks = mod


def run_kernel(x, W, trace=False):
    """Returns (output [256,10,16,1] f32, BassKernelResults)."""
    if trace:
        _ensure_ntff_hook()
    nc = _get_program()
    in_maps = _prep_inputs(x, W)
    bkr = run_bass_kernel_spmd(nc, in_maps, list(range(NCORES)), trace=trace)
    v = np.concatenate([bkr.results[c]["vout"] for c in range(NCORES)], axis=0)
    return v[..., None].astype(np.float32), bkr


def kernel(x, W):
    try:
        out, _ = run_kernel(x, W, trace=False)
    except Exception:
        # one retry: a previous aborted run can leave a core wedged once
        out, _ = run_kernel(x, W, trace=False)
    return out
